# revision 7
# baseline (speedup 1.0000x reference)
"""Trainium2 Bass kernel: diagonal complex SSM scan via Toeplitz block-matmuls.

out[t, d] = z_d * out[t-1, d] + x[t, d],  z_d = exp(-exp(size_d) + i*theta_d)

T-scheme v3b: PE computes block-local complex scans via per-tile Toeplitz
weights; DVE handles only the 16x-decimated carry scan; DMAs are consolidated
into slab transfers (the v3a lesson: 300 small DMAs serialized 180us on the
sync queue).  See kernel.py history for the derivation.
"""

import os
import sys

import numpy as np

for _p in ("/opt/trn_rl_repo", "/root/.axon_site/_ro/trn_rl_repo"):
    if os.path.isdir(_p) and _p not in sys.path:
        sys.path.append(_p)

import concourse.bacc as bacc
import concourse.mybir as mybir
from concourse import bass_utils
from concourse.tile import TileContext

T = 8192
D = 2048
NCORES = 8
DS = D // NCORES          # 256 channels per core
B = 16                    # block length (Toeplitz size)
J = T // B                # 512 blocks
CHT = 128 // B            # 8 channels per tile
NT = DS // CHT            # 32 tiles per core
NB = 2                    # carry batches (128 channels each)
TPB = NT // NB            # 16 tiles per batch
QUAD = 4                  # tiles per psum/drain/DMA group
NQ = NT // QUAD           # 8 quads
F32 = mybir.dt.float32
F16 = mybir.dt.float16

_PROGRAM = None


def _build_program():
    nc = bacc.Bacc("TRN2", target_bir_lowering=False)

    xP = nc.dram_tensor("xP", (128, NT * J), F16, kind="ExternalInput")
    mre = nc.dram_tensor("mre", (128, NT * 128), F16, kind="ExternalInput")
    mim = nc.dram_tensor("mim", (128, NT * 128), F16, kind="ExternalInput")
    w1 = nc.dram_tensor("w1", (128, NT * 128), F16, kind="ExternalInput")
    w2 = nc.dram_tensor("w2", (128, NT * 128), F16, kind="ExternalInput")
    cosB = nc.dram_tensor("cosB", (DS, J), F16, kind="ExternalInput")
    sinB = nc.dram_tensor("sinB", (DS, J), F16, kind="ExternalInput")
    rB = nc.dram_tensor("rB", (DS, 1), F32, kind="ExternalInput")
    eye = nc.dram_tensor("eye", (128, 128), F16, kind="ExternalInput")
    o_re = nc.dram_tensor("o_re", (128, NT * J), F16, kind="ExternalOutput")
    o_im = nc.dram_tensor("o_im", (128, NT * J), F16, kind="ExternalOutput")

    mult = mybir.AluOpType.mult
    add = mybir.AluOpType.add
    subtract = mybir.AluOpType.subtract

    with TileContext(nc) as tc:
        with tc.tile_pool(name="tabs", bufs=1) as tpool, \
             tc.tile_pool(name="xz", bufs=4) as xpool, \
             tc.tile_pool(name="az", bufs=1) as apool, \
             tc.tile_pool(name="cz", bufs=1) as cpool, \
             tc.tile_pool(name="oz", bufs=2) as opool, \
             tc.tile_pool(name="pA", bufs=1, space="PSUM") as ppA, \
             tc.tile_pool(name="pQ", bufs=2, space="PSUM") as ppQ:
            # PSUM budget: pA holds tags psA0/psA1 at [128,1024] (2 banks
            # each, bufs=1) and pQ tags qre/qim at [128,512] (1 bank, bufs=2)
            # -> 4 + 4 = 8 banks.

            # ---------------- prologue DMAs (slab-sized) -------------------
            xq = [None] * NQ                      # [128, QUAD*J] x slabs
            # lead-in split: first tile's x + first quad's weights land first
            # so the PE starts ~6us earlier.
            xq[0] = xpool.tile([128, QUAD * J], F16, name="xq", tag="xq")
            mre_s = tpool.tile([128, NT * 128], F16, name="mre_s")
            mim_s = tpool.tile([128, NT * 128], F16, name="mim_s")
            nc.sync.dma_start(xq[0][:, 0:J], xP[:, 0:J])
            nc.sync.dma_start(mre_s[:, 0:QUAD * 128], mre[:, 0:QUAD * 128])
            nc.sync.dma_start(mim_s[:, 0:QUAD * 128], mim[:, 0:QUAD * 128])
            nc.sync.dma_start(xq[0][:, J:QUAD * J], xP[:, J:QUAD * J])
            nc.sync.dma_start(mre_s[:, QUAD * 128:], mre[:, QUAD * 128:])
            xq[1] = xpool.tile([128, QUAD * J], F16, name="xq", tag="xq")
            nc.sync.dma_start(xq[1][:], xP[:, QUAD * J:2 * QUAD * J])
            nc.sync.dma_start(mim_s[:, QUAD * 128:], mim[:, QUAD * 128:])
            eye_t = tpool.tile([128, 128], F16, name="eye_t")
            nc.sync.dma_start(eye_t[:], eye[:])
            cosB_t, sinB_t, rB_t = [], [], []
            for g in range(NB):
                ct = tpool.tile([128, J], F16, name=f"cosB{g}")
                st = tpool.tile([128, J], F16, name=f"sinB{g}")
                rt = tpool.tile([128, 1], F32, name=f"rB{g}")
                nc.sync.dma_start(ct[:], cosB[g * 128:(g + 1) * 128, :])
                nc.sync.dma_start(st[:], sinB[g * 128:(g + 1) * 128, :])
                nc.sync.dma_start(rt[:], rB[g * 128:(g + 1) * 128, :])
                cosB_t.append(ct)
                sinB_t.append(st)
                rB_t.append(rt)
            w1_s = tpool.tile([128, NT * 128], F16, name="w1_s")
            w2_s = tpool.tile([128, NT * 128], F16, name="w2_s")
            nc.sync.dma_start(w1_s[:], w1[:, :])
            nc.sync.dma_start(w2_s[:], w2[:, :])

            def wsl(slab, t):
                return slab[:, t * 128:(t + 1) * 128]

            # A[quad][comp] = fp16 [128, QUAD*J] tile (4 tiles side by side)
            A = [[None, None] for _ in range(NQ)]
            # E lastrow tiles, one per (batch, comp), filled by per-quad
            # gathers emitted right after each quad's drain.
            Etiles = [[cpool.tile([128, J], F16, name=f"E{g}c{c}")
                       for c in range(2)] for g in range(NB)]

            def pass1_quad(m):
                # pair-grain psum with separate re/im tags: while Act/DVE
                # drain one comp's [128,1024] psum, PE fills the other tag.
                for pf in (m + 2, m + 3):
                    if pf < NQ and xq[pf] is None:
                        nx = xpool.tile([128, QUAD * J], F16, name="xq",
                                        tag="xq")
                        nc.sync.dma_start(
                            nx[:], xP[:, pf * QUAD * J:(pf + 1) * QUAD * J])
                        xq[pf] = nx
                g = (QUAD * m) // TPB
                ats = []
                for comp in range(2):
                    at = apool.tile([128, QUAD * J], F16, name=f"A{m}c{comp}")
                    A[m][comp] = at
                    ats.append(at)
                for half in range(2):       # tiles (2*half, 2*half+1)
                    for comp in range(2):
                        mslab = mre_s if comp == 0 else mim_s
                        ps = ppA.tile([128, 2 * J], F32, name="psA",
                                      tag=f"psA{comp}")
                        for kk in range(2):
                            k = 2 * half + kk
                            t = QUAD * m + k
                            nc.tensor.matmul(ps[:, kk * J:(kk + 1) * J],
                                             wsl(mslab, t),
                                             xq[m][:, k * J:(k + 1) * J],
                                             start=True, stop=True)
                        dst = ats[comp][:, 2 * half * J:(2 * half + 2) * J]
                        # re drains on Act, im on DVE — except the last two
                        # quads' im drains go to Act so DVE can enter the
                        # carries(1) chain the moment E(b1) lands.
                        if comp == 0 or m >= NQ - 2:
                            nc.scalar.copy(dst, ps[:])
                        else:
                            nc.vector.tensor_copy(dst, ps[:])
                for comp in range(2):
                    # gather this quad's lastrows into E
                    et = Etiles[g][comp]
                    mq = m - g * (NQ // NB)
                    for k in range(QUAD):
                        kk = mq * QUAD + k
                        eng = nc.gpsimd if k % 2 == 0 else nc.sync
                        eng.dma_start(et[kk * CHT:(kk + 1) * CHT, :],
                                      ats[comp][120:128, k * J:(k + 1) * J])

            Cstack = [[None, None] for _ in range(NB)]  # [batch][half 64ch]

            def carries(g):
                E = Etiles[g]
                ct, st = cosB_t[g], sinB_t[g]
                ta = cpool.tile([128, J], F16, name=f"ta{g}")
                tb = cpool.tile([128, J], F16, name=f"tb{g}")
                ehre = cpool.tile([128, J], F16, name=f"ehre{g}")
                ehim = cpool.tile([128, J], F16, name=f"ehim{g}")
                # twist by e^{-i theta B j}
                nc.vector.tensor_mul(ta[:], ct[:], E[0][:])
                nc.vector.tensor_mul(tb[:], st[:], E[1][:])
                nc.vector.tensor_tensor(ehre[:], ta[:], tb[:], op=add)
                nc.vector.tensor_mul(ta[:], ct[:], E[1][:])
                nc.vector.tensor_mul(tb[:], st[:], E[0][:])
                nc.vector.tensor_tensor(ehim[:], ta[:], tb[:], op=subtract)
                # real scans with multiplier r^B (broadcast [P,1])
                chre = cpool.tile([128, J], F16, name=f"chre{g}")
                chim = cpool.tile([128, J], F16, name=f"chim{g}")
                rb = rB_t[g][:].broadcast_to([128, J])
                nc.vector.tensor_tensor_scan(chre[:], rb, ehre[:], 0.0,
                                             op0=mult, op1=add)
                nc.vector.tensor_tensor_scan(chim[:], rb, ehim[:], 0.0,
                                             op0=mult, op1=add)
                # untwist by e^{+i theta B j}, shifted one block right
                cre = cpool.tile([128, J], F16, name=f"cre{g}")
                cim = cpool.tile([128, J], F16, name=f"cim{g}")
                nc.vector.memset(cre[:, 0:1], 0.0)
                nc.vector.memset(cim[:, 0:1], 0.0)
                sl = slice(0, J - 1)
                sh = slice(1, J)
                nc.vector.tensor_mul(ta[:, sl], ct[:, sl], chre[:, sl])
                nc.vector.tensor_mul(tb[:, sl], st[:, sl], chim[:, sl])
                nc.vector.tensor_tensor(cre[:, sh], ta[:, sl], tb[:, sl],
                                        op=subtract)
                nc.vector.tensor_mul(ta[:, sl], ct[:, sl], chim[:, sl])
                nc.vector.tensor_mul(tb[:, sl], st[:, sl], chre[:, sl])
                nc.vector.tensor_tensor(cim[:, sh], ta[:, sl], tb[:, sl],
                                        op=add)
                # stacked [Cre(64ch); Cim(64ch)] moving tiles for inject
                for h in range(2):
                    stk = cpool.tile([128, J], F16, name=f"stk{g}h{h}")
                    nc.gpsimd.dma_start(stk[0:64, :],
                                        cre[h * 64:(h + 1) * 64, :])
                    nc.sync.dma_start(stk[64:128, :],
                                      cim[h * 64:(h + 1) * 64, :])
                    Cstack[g][h] = stk

            def pass2_quad(m):
                g = (QUAD * m) // TPB
                ores = opool.tile([128, QUAD * J], F16, name="ore", tag="ore")
                oims = opool.tile([128, QUAD * J], F16, name="oim", tag="oim")
                for k in range(QUAD):
                    t = QUAD * m + k
                    h = (t % TPB) // (TPB // 2)
                    stk = Cstack[g][h]
                    ksl = slice(k * J, (k + 1) * J)
                    # comp re: inject + A merged on PE, drained by Act
                    qre = ppQ.tile([128, J], F32, name="qre", tag="qre")
                    nc.tensor.matmul(qre[:], wsl(w1_s, t), stk[:],
                                     start=True, stop=False)
                    nc.tensor.matmul(qre[:], eye_t[:], A[m][0][:, ksl],
                                     start=False, stop=True)
                    nc.scalar.copy(ores[:, ksl], qre[:])
                    # comp im: inject on PE, merged with A on DVE
                    qim = ppQ.tile([128, J], F32, name="qim", tag="qim")
                    nc.tensor.matmul(qim[:], wsl(w2_s, t), stk[:],
                                     start=True, stop=True)
                    nc.vector.tensor_tensor(oims[:, ksl], A[m][1][:, ksl],
                                            qim[:], op=add)
                nc.gpsimd.dma_start(
                    o_re[:, QUAD * m * J:(QUAD * m + QUAD) * J], ores[:])
                nc.sync.dma_start(
                    o_im[:, QUAD * m * J:(QUAD * m + QUAD) * J], oims[:])

            # ---------------- schedule -------------------------------------
            # minimal interleave (measured best): pass 1 finishes ASAP; only
            # one pass-2 quad overlaps the pass-1 tail, the rest follow the
            # carries.
            for m in range(NQ):
                pass1_quad(m)
                if m == NQ // NB:
                    carries(0)
            carries(1)
            for m in range(NQ):
                pass2_quad(m)

    nc.compile()
    return nc


def _get_program():
    global _PROGRAM
    if _PROGRAM is None:
        _PROGRAM = _build_program()
    return _PROGRAM


def _host_prep(x, size, theta):
    size64 = np.asarray(size, np.float64)
    theta64 = np.asarray(theta, np.float64)
    r = np.exp(-np.exp(size64))
    z = r * np.exp(1j * theta64)                      # [D] complex128

    x16 = np.asarray(x, np.float32).astype(np.float16)

    zp = np.empty((B + 1, D), np.complex128)          # z^0 .. z^B
    zp[0] = 1.0
    for k in range(1, B + 1):
        zp[k] = zp[k - 1] * z

    jj = np.arange(J, dtype=np.float64)
    angB = theta64[:, None] * (B * jj[None, :])
    cosBf = np.cos(angB).astype(np.float16)
    sinBf = np.sin(angB).astype(np.float16)
    rBf = (r ** B).astype(np.float32)[:, None]
    eyef = np.eye(128, dtype=np.float16)

    # Toeplitz structure shared by all tiles: for s<=q,
    # M[s*CHT+dl, q*CHT+dl] = z_d^{q-s}  (p = q*CHT + dl, q-major)
    s_idx, q_idx = np.meshgrid(np.arange(B), np.arange(B), indexing="ij")
    low = (q_idx >= s_idx)
    pw = np.where(low, q_idx - s_idx, 0)              # [B, B]

    in_maps = []
    for cidx in range(NCORES):
        ch0 = cidx * DS
        xPc = np.empty((128, NT * J), np.float16)
        mrec = np.zeros((128, NT * 128), np.float16)
        mimc = np.zeros((128, NT * 128), np.float16)
        w1c = np.zeros((128, NT * 128), np.float16)
        w2c = np.zeros((128, NT * 128), np.float16)
        for t in range(NT):
            chs = ch0 + CHT * t + np.arange(CHT)
            xi = x16[:, chs].reshape(J, B, CHT)       # [j, q, dl]
            xPc[:, t * J:(t + 1) * J] = xi.transpose(1, 2, 0).reshape(128, J)
            zt = zp[:, chs]                            # [B+1, CHT] powers
            zv = zt[pw.reshape(-1)].reshape(B, B, CHT)  # [s, q, dl]
            zv = np.where(low[:, :, None], zv, 0)
            Mre = np.zeros((128, 128), np.float64)
            Mim = np.zeros((128, 128), np.float64)
            for dl in range(CHT):
                Mre[dl::CHT, dl::CHT] = zv[:, :, dl].real
                Mim[dl::CHT, dl::CHT] = zv[:, :, dl].imag
            mrec[:, t * 128:(t + 1) * 128] = Mre.astype(np.float16)
            mimc[:, t * 128:(t + 1) * 128] = Mim.astype(np.float16)
            gg = t // TPB
            hh = (t % TPB) // (TPB // 2)
            W1 = np.zeros((128, 128), np.float64)
            W2 = np.zeros((128, 128), np.float64)
            for dl in range(CHT):
                d_in_core = CHT * t + dl
                row = (d_in_core - 128 * gg) - 64 * hh
                zq = zp[1:B + 1, ch0 + d_in_core]      # z^{q+1}
                cols = np.arange(B) * CHT + dl
                W1[row, cols] = zq.real
                W1[64 + row, cols] = -zq.imag
                W2[row, cols] = zq.imag
                W2[64 + row, cols] = zq.real
            w1c[:, t * 128:(t + 1) * 128] = W1.astype(np.float16)
            w2c[:, t * 128:(t + 1) * 128] = W2.astype(np.float16)
        sl = slice(ch0, ch0 + DS)
        in_maps.append({
            "xP": xPc, "mre": mrec, "mim": mimc, "w1": w1c, "w2": w2c,
            "cosB": np.ascontiguousarray(cosBf[sl]),
            "sinB": np.ascontiguousarray(sinBf[sl]),
            "rB": np.ascontiguousarray(rBf[sl]),
            "eye": eyef,
        })
    return in_maps


def _assemble(results):
    out = np.empty((T, D), np.complex64)
    for cidx, res in enumerate(results):
        ch0 = cidx * DS
        re = res["o_re"].astype(np.float32).reshape(B, CHT, NT, J)
        im = res["o_im"].astype(np.float32).reshape(B, CHT, NT, J)
        v = (re + 1j * im).transpose(3, 0, 2, 1)      # [j, q, t, dl]
        out[:, ch0:ch0 + DS] = v.reshape(T, DS)
    return out


def run(x, size, theta, trace=False, **spmd_kwargs):
    nc = _get_program()
    in_maps = _host_prep(x, size, theta)
    res = bass_utils.run_bass_kernel_spmd(
        nc, in_maps, core_ids=list(range(NCORES)), trace=trace, **spmd_kwargs)
    return _assemble(res.results), res


def kernel(x, size, theta):
    out, _ = run(x, size, theta, trace=False)
    return out
